# revision 20
# baseline (speedup 1.0000x reference)
"""MoE ConditionalFeedForward (SwiGLU, T=2048 D=1024 I=4096 E=8 K=2) on 8 TRN2 cores.

Strategy: expert-parallel, one expert per NeuronCore. Routing/gather happens on
host (numpy): for each expert e, collect the unique tokens routed to it, merge
the two top-k gate weights, and ship the gathered tokens transposed plus that
expert's three weight matrices, pre-packed so every device DMA is a fully
linear HBM read. Each core computes
  y_e = (silu(x @ w1e^T) * (x @ w3e^T)) @ w2e^T * gate
for its <=CAP tokens; the host scatter-adds the 8 partials into [T, D].

Device kernel (per core), v5 — bf16 operands (PE rate identical to fp32r,
half the HBM traffic; rel err ~4e-3 vs the 2e-2 gate), tuned from traces:
  queues: DMA triggers cost ~0.7us of issuing-engine queue time each, so they
          live on sync + gpsimd; ACT/DVE run pure compute mid-stream so the
          silu/mul WAR chain never blocks the PE (exception: two ramp loads
          ride the scalar ring before its first silu).
  warmup: ~45 tiny matmuls on a memset tile while the first DMAs land, so the
          PE HAM clock-gate is at 8/8 (2.4 GHz) when real work starts.
  ncol:   matmul free dim is the actual max tokens/expert (rounded to 8),
          not CAP — the NEFF is rebuilt if a routing ever exceeds it.
  layer 1: per i-tile, accumulate 8 K=128 steps into two PSUM banks (h1, h3),
           then ACT silu + DVE multiply into an SBUF hT tile laid out [i, t]
           so it feeds layer 2 as lhsT directly.
  layer 2: two d-halves (dc), one PSUM bank per token-tile (tile-granular
           dependency tracking — a shared tile would serialize drains against
           the next tile's matmuls). w2 streams as 4-i-tile quads; in the
           final quad the matmuls run token-tile-major in 2+2 groups so each
           tile's gate-scaled drain (split ACT/DVE) and output DMA overlap
           the remaining matmuls — the kernel tail is one short copy + DMA.
"""

import math
import os
import sys
import time
import types

for _p in ("/opt/trn_rl_repo", "/opt/pypackages"):
    if _p not in sys.path:
        sys.path.append(_p)

import numpy as np
import ml_dtypes

# antenv.axon_hooks is absent from this image; run_bass_kernel_spmd imports it
# unconditionally when tracing is requested (BASS_TRACE=1). Provide the
# documented shim so profiling works when asked for and degrades to a no-op
# otherwise. No-op if a real antenv.axon_hooks exists.
def _ensure_ntff_hook():
    try:
        import antenv
    except ImportError:
        return
    try:
        import antenv.axon_hooks  # noqa: F401
        return
    except ImportError:
        pass
    mod = types.ModuleType("antenv.axon_hooks")
    mod._hook = None

    def set_axon_ntff_profile_hook(h):
        mod._hook = h

    def get_axon_ntff_profile_hook():
        if mod._hook is None:
            try:
                from trn_agent_boot.trn_boot import _ntff_profile_via_ctypes

                mod._hook = _ntff_profile_via_ctypes("/opt/axon/libaxon_pjrt.so")
            except Exception:
                mod._hook = None
        return mod._hook

    mod.set_axon_ntff_profile_hook = set_axon_ntff_profile_hook
    mod.get_axon_ntff_profile_hook = get_axon_ntff_profile_hook
    sys.modules["antenv.axon_hooks"] = mod
    antenv.axon_hooks = mod


_ensure_ntff_hook()

import concourse.bacc as bacc
import concourse.tile as tile
from concourse import mybir
from concourse.bass_utils import run_bass_kernel_spmd

T, D, I, E, TOPK = 2048, 1024, 4096, 8, 2
N_CORES = 8
CAP = 512            # tokens per expert per pass (multiple of 128, <=512)
DT = D // 128        # 8 contraction steps for layer 1
NI = I // 128        # 32 intermediate tiles
NQ = 8               # layer-2 w2 quads per d-half (4 i-tiles each)
F32 = mybir.dt.float32
BF16 = mybir.dt.bfloat16
BF16_NP = ml_dtypes.bfloat16
N_WARM = 68          # tiny matmuls to lift the HAM clock gate during the ramp

_NCS = {}            # compiled Bass modules, keyed by ncol
_WCACHE = {}         # packed per-expert weights, keyed on input identity
LAST_RESULTS = None  # BassKernelResults of the most recent SPMD run


def _build_nc(ncol, sim_act=False):
    # sim_act: CoreSim lacks Silu; emit sigmoid + extra multiply instead
    # (same math) so the program can be validated in simulation.
    nt = (ncol + 127) // 128          # token tiles
    last_rows = ncol - 128 * (nt - 1)  # rows in the final token tile
    nc = bacc.Bacc(
        "TRN2", target_bir_lowering=False, debug=False, num_devices=N_CORES
    )
    # Packed layouts (see _pack_weights): every DMA below reads HBM linearly.
    xt_d = nc.dram_tensor("xt", [DT, 128, CAP], BF16, kind="ExternalInput").ap()
    g_d = nc.dram_tensor("g", [CAP], F32, kind="ExternalInput").ap()
    w13p_d = nc.dram_tensor(
        "w13p", [NI, 2, 128, DT, 128], BF16, kind="ExternalInput"
    ).ap()
    w2p_d = nc.dram_tensor("w2p", [2, NI, 128, 512], BF16, kind="ExternalInput").ap()
    y_d = nc.dram_tensor("y", [CAP, D], F32, kind="ExternalOutput").ap()

    with tile.TileContext(nc) as tc:
        with (
            tc.tile_pool(name="consts", bufs=1) as const_pool,
            tc.tile_pool(name="w13", bufs=7) as w13_pool,
            tc.tile_pool(name="w2", bufs=10) as w2_pool,
            tc.tile_pool(name="h", bufs=1) as h_pool,
            tc.tile_pool(name="tmp", bufs=2) as tmp_pool,
            tc.tile_pool(name="yout", bufs=4) as out_pool,
        ):
            # PE warmup: the HAM clock gate needs ~3.4us of sustained PE
            # activity to move 1.2 -> 2.4 GHz. Run tiny matmuls on a memset
            # tile (no DMA dependency) while the first real tiles stream in.
            warm_sb = const_pool.tile([128, 64], BF16)
            nc.vector.memset(warm_sb[:], 0.0)
            warm_pool = tc.alloc_tile_pool(name="warm", bufs=1, space="PSUM")
            warm_ps = warm_pool.tile([64, 64], F32)
            for _ in range(N_WARM):
                nc.tensor.matmul(
                    warm_ps[:], warm_sb[:, :64], warm_sb[:, :64],
                    start=True, stop=True,
                )
            warm_pool.release()

            # Resident activations: x^T as 8 [128, CAP] d-tiles. The SDMA
            # engines round-robin all queued rings at packet granularity, so
            # aggregate HBM bandwidth (~300 GB/s) is the ramp constraint:
            # every ramp-critical transfer goes on the sync ring, in exact
            # consumption order ([xt lo, w13 it0, xt hi, w13 it1, it2, ...]),
            # so FIFO serialization gives each one full bandwidth in turn.
            xt_sb = const_pool.tile([128, DT, CAP], BF16)
            nc.sync.dma_start(
                xt_sb[:, 0:4, :], xt_d[0:4].rearrange("a p c -> p a c")
            )
            g_sb = const_pool.tile([128, nt], F32)
            # Gates: tiny gpsimd load, needed only at the layer-2 drain.
            nc.gpsimd.dma_start(
                g_sb[:], g_d[:nt * 128].rearrange("(a p) -> p a", p=128)
            )

            # hT[i, t] — layer-1 output (bf16), transposed so it is lhsT for
            # layer 2.
            hT = h_pool.tile([128, NI, ncol], BF16)

            # ps2 allocated first so ps1 can be released in LIFO order after
            # layer 1 while ps2's banks stay live for both layer-2 halves.
            ps2_pool = tc.alloc_tile_pool(name="ps2", bufs=1, space="PSUM")
            ps1_pool = tc.alloc_tile_pool(name="ps1", bufs=2, space="PSUM")
            w2_tiles = {}
            for it in range(NI):
                w13_t = w13_pool.tile([128, 2, DT, 128], BF16, tag="w13")
                # One linear 512 KB DMA per w13 tile, all on the sync ring in
                # consumption order (see ramp note above). ACT and DVE stay
                # trigger-free so the silu/mul WAR chain never blocks the PE.
                nc.sync.dma_start(
                    w13_t[:], w13p_d[it].rearrange("m p a c -> p m a c")
                )
                if it == 0:
                    # xt d-tiles 4..7 land behind w13-it0, just before their
                    # first matmul (~8 matmuls in) needs them.
                    nc.sync.dma_start(
                        xt_sb[:, 4:8, :], xt_d[4:8].rearrange("a p c -> p a c")
                    )
                if it >= 16 and it % 2 == 0:
                    # w2 dc0 quads prefetch on the idle gpsimd ring once the
                    # ramp is past; issuing them any earlier steals HBM
                    # bandwidth from the ramp-critical w13 stream.
                    iq = (it - 16) // 2
                    w2_t = w2_pool.tile([128, 4, 512], BF16, tag="w2")
                    nc.gpsimd.dma_start(
                        w2_t[:],
                        w2p_d[0, iq * 4:(iq + 1) * 4].rearrange("a p n -> p a n"),
                    )
                    w2_tiles[(0, iq)] = w2_t
                h1_ps = ps1_pool.tile([128, ncol], F32, tag="h1")
                h3_ps = ps1_pool.tile([128, ncol], F32, tag="h3")
                for dt_i in range(DT):
                    nc.tensor.matmul(
                        h1_ps[:],
                        w13_t[:, 0, dt_i, :],
                        xt_sb[:, dt_i, :ncol],
                        start=(dt_i == 0),
                        stop=(dt_i == DT - 1),
                    )
                for dt_i in range(DT):
                    nc.tensor.matmul(
                        h3_ps[:],
                        w13_t[:, 1, dt_i, :],
                        xt_sb[:, dt_i, :ncol],
                        start=(dt_i == 0),
                        stop=(dt_i == DT - 1),
                    )
                s_sb = tmp_pool.tile([128, ncol], F32)
                if sim_act:
                    nc.scalar.activation(
                        s_sb[:], h1_ps[:], mybir.ActivationFunctionType.Sigmoid
                    )
                    nc.vector.tensor_mul(s_sb[:], s_sb[:], h1_ps[:])
                else:
                    nc.scalar.activation(
                        s_sb[:], h1_ps[:], mybir.ActivationFunctionType.Silu
                    )
                nc.vector.tensor_mul(hT[:, it, :], s_sb[:], h3_ps[:])

            ps1_pool.release()

            # Layer 2, one d-half (512 cols) at a time: one PSUM bank per
            # token-tile accumulates y[t, dc] over the 32 i-tiles. The final
            # quad runs token-tile-major in 2+2 i-groups so each tile's drain
            # and output DMA overlap the remaining tiles' matmuls.
            def rows(tt):
                return last_rows if tt == nt - 1 else 128

            def l2mm(y_ps, tt, it, w2_t, j):
                nc.tensor.matmul(
                    y_ps[tt][:rows(tt), :],
                    hT[:, it, tt * 128:tt * 128 + rows(tt)],
                    w2_t[:, j, :],
                    start=(it == 0),
                    stop=(it == NI - 1),
                )

            def drain(dc, tt, y_ps):
                r = rows(tt)
                y_sb = out_pool.tile([128, 512], F32)
                src = y_ps[tt]
                gs = g_sb[:r, tt:tt + 1]
                lo, hi = (slice(0, 256), slice(256, 512))
                if tt % 2:
                    lo, hi = hi, lo
                nc.scalar.activation(
                    y_sb[:r, lo], src[:r, lo],
                    mybir.ActivationFunctionType.Copy, scale=gs,
                )
                nc.vector.tensor_scalar_mul(y_sb[:r, hi], src[:r, hi], gs)
                dst = y_d[tt * 128:tt * 128 + r, dc * 512:(dc + 1) * 512]
                if dc == 1 and tt == nt - 1:
                    # Kernel-tail DMA: split across both HWDGE rings.
                    nc.sync.dma_start(dst[:, :256], y_sb[:r, :256])
                    nc.scalar.dma_start(dst[:, 256:], y_sb[:r, 256:])
                else:
                    eng = nc.gpsimd if dc == 0 else nc.sync
                    eng.dma_start(dst, y_sb[:r, :])

            for dc in range(2):
                # One PSUM tile per token-tile: tile-granular dependency
                # tracking would otherwise serialize each tile's drain
                # against the next tile's matmuls.
                y_ps = [
                    ps2_pool.tile([128, 512], F32, tag=f"y{tt}", name=f"y_ps{tt}")
                    for tt in range(nt)
                ]
                for iq in range(NQ):
                    if dc == 0:
                        # Loaded during layer 1 on the gpsimd ring.
                        w2_t = w2_tiles[(0, iq)]
                    else:
                        # dc1 quads on sync: they queue behind the w13 loads
                        # and prefetch fully during dc0's matmuls.
                        w2_t = w2_pool.tile([128, 4, 512], BF16, tag="w2")
                        nc.sync.dma_start(
                            w2_t[:],
                            w2p_d[1, iq * 4:(iq + 1) * 4].rearrange("a p n -> p a n"),
                        )
                    if iq < NQ - 1:
                        for j in range(4):
                            for tt in range(nt):
                                l2mm(y_ps, tt, iq * 4 + j, w2_t, j)
                    else:
                        # Final quad: 2+2 i-groups, token-tile-major, with
                        # each tile's drain right after its last matmul.
                        for tt in range(nt):
                            for j in range(2):
                                l2mm(y_ps, tt, iq * 4 + j, w2_t, j)
                        for tt in range(nt):
                            for j in range(2, 4):
                                l2mm(y_ps, tt, iq * 4 + j, w2_t, j)
                            drain(dc, tt, y_ps)
            ps2_pool.release()

    nc.compile()
    return nc


def _pack_weights(w1, w2, w3):
    """Per-expert device layouts (bf16), all linear HBM reads:
    w13p[it, m, p, dt, c] = wm[it*128+c, dt*128+p]  (i.e. w.T tiled for lhsT)
    w2p[dc, it, p, n] = w2[dc*512+n, it*128+p]      (w2.T tiled by d-half)."""
    key = tuple((a.ctypes.data, a.shape) for a in (w1, w2, w3))
    if _WCACHE.get("key") == key:
        return _WCACHE["maps"]
    maps = []
    for e in range(E):
        w13p = np.empty((NI, 2, 128, DT, 128), dtype=BF16_NP)
        w13p[:, 0] = w1[e].reshape(NI, 128, DT, 128).transpose(0, 3, 2, 1)
        w13p[:, 1] = w3[e].reshape(NI, 128, DT, 128).transpose(0, 3, 2, 1)
        w2p = np.ascontiguousarray(
            w2[e].T.reshape(NI, 128, 2, 512).transpose(2, 0, 1, 3)
        ).astype(BF16_NP)
        maps.append({"w13p": w13p, "w2p": w2p})
    _WCACHE["key"] = key
    _WCACHE["maps"] = maps
    return maps


def kernel(x, expert_indices, expert_weights, w1, w2, w3):
    global LAST_RESULTS
    x = np.ascontiguousarray(np.asarray(x, dtype=np.float32))
    idx = np.asarray(expert_indices)
    ew = np.asarray(expert_weights, dtype=np.float32)
    w1 = np.ascontiguousarray(np.asarray(w1, dtype=np.float32))
    w2 = np.ascontiguousarray(np.asarray(w2, dtype=np.float32))
    w3 = np.ascontiguousarray(np.asarray(w3, dtype=np.float32))

    # Host routing: unique tokens per expert, with both top-k gate weights of a
    # token merged (a token picking the same expert twice gets the summed gate).
    tok_lists, gate_lists = [], []
    for e in range(E):
        m = idx == e
        sel = np.nonzero(m.any(axis=1))[0]
        tok_lists.append(sel)
        gate_lists.append((ew * m).sum(axis=1)[sel].astype(np.float32))

    weight_maps = _pack_weights(w1, w2, w3)

    n_max = max(len(s) for s in tok_lists)
    n_pass = max(1, math.ceil(n_max / CAP))
    # Matmul free dim: actual max tokens this pass (rounded to 8), so the PE
    # streams no padded columns. Compiled once per distinct ncol.
    ncol = min(CAP, -(-min(n_max, CAP) // 8) * 8)
    if ncol not in _NCS:
        _NCS[ncol] = _build_nc(ncol)
    nc_mod = _NCS[ncol]

    out = np.zeros((T, D), dtype=np.float32)
    trace = bool(os.environ.get("BASS_TRACE"))
    for p in range(n_pass):
        in_maps = []
        chunks = []
        for e in range(E):
            sel = tok_lists[e][p * CAP:(p + 1) * CAP]
            g = gate_lists[e][p * CAP:(p + 1) * CAP]
            chunks.append(sel)
            xt = np.zeros((D, CAP), dtype=np.float32)
            if len(sel):
                xt[:, :len(sel)] = x[sel].T
            xt = xt.astype(BF16_NP).reshape(DT, 128, CAP)
            g_pad = np.zeros((CAP,), dtype=np.float32)
            g_pad[:len(sel)] = g
            in_maps.append({"xt": xt, "g": g_pad, **weight_maps[e]})
        # Rare transient NRT_EXEC_UNIT_UNRECOVERABLE errors have been observed
        # on the first execution of a fresh NEFF; a straight retry recovers.
        last_exc = None
        for attempt in range(3):
            try:
                LAST_RESULTS = run_bass_kernel_spmd(
                    nc_mod, in_maps, core_ids=list(range(N_CORES)),
                    trace=trace and attempt == 0,
                )
                break
            except Exception as exc:  # noqa: BLE001
                last_exc = exc
                time.sleep(3)
        else:
            raise last_exc
        for e in range(E):
            sel = chunks[e]
            if len(sel):
                out[sel] += LAST_RESULTS.results[e]["y"][:len(sel)]
    return out


# revision 21
# speedup vs baseline: 1.0192x; 1.0192x over previous
"""MoE ConditionalFeedForward (SwiGLU, T=2048 D=1024 I=4096 E=8 K=2) on 8 TRN2 cores.

Strategy: expert-parallel, one expert per NeuronCore. Routing/gather happens on
host (numpy): for each expert e, collect the unique tokens routed to it, merge
the two top-k gate weights, and ship the gathered tokens transposed plus that
expert's three weight matrices, pre-packed so every device DMA is a fully
linear HBM read. Each core computes
  y_e = (silu(x @ w1e^T) * (x @ w3e^T)) @ w2e^T * gate
for its <=CAP tokens; the host scatter-adds the 8 partials into [T, D].

Device kernel (per core), v5 — bf16 operands (PE rate identical to fp32r,
half the HBM traffic; rel err ~4e-3 vs the 2e-2 gate), tuned from traces:
  queues: DMA triggers cost ~0.7us of issuing-engine queue time each, so they
          live on sync + gpsimd; ACT/DVE run pure compute mid-stream so the
          silu/mul WAR chain never blocks the PE (exception: two ramp loads
          ride the scalar ring before its first silu).
  warmup: ~45 tiny matmuls on a memset tile while the first DMAs land, so the
          PE HAM clock-gate is at 8/8 (2.4 GHz) when real work starts.
  ncol:   matmul free dim is the actual max tokens/expert (rounded to 8),
          not CAP — the NEFF is rebuilt if a routing ever exceeds it.
  layer 1: per i-tile, accumulate 8 K=128 steps into two PSUM banks (h1, h3),
           then ACT silu + DVE multiply into an SBUF hT tile laid out [i, t]
           so it feeds layer 2 as lhsT directly.
  layer 2: two d-halves (dc), one PSUM bank per token-tile (tile-granular
           dependency tracking — a shared tile would serialize drains against
           the next tile's matmuls). w2 streams as 4-i-tile quads; in the
           final quad the matmuls run token-tile-major in 2+2 groups so each
           tile's gate-scaled drain (split ACT/DVE) and output DMA overlap
           the remaining matmuls — the kernel tail is one short copy + DMA.
"""

import math
import os
import sys
import time
import types

for _p in ("/opt/trn_rl_repo", "/opt/pypackages"):
    if _p not in sys.path:
        sys.path.append(_p)

import numpy as np
import ml_dtypes

# antenv.axon_hooks is absent from this image; run_bass_kernel_spmd imports it
# unconditionally when tracing is requested (BASS_TRACE=1). Provide the
# documented shim so profiling works when asked for and degrades to a no-op
# otherwise. No-op if a real antenv.axon_hooks exists.
def _ensure_ntff_hook():
    try:
        import antenv
    except ImportError:
        return
    try:
        import antenv.axon_hooks  # noqa: F401
        return
    except ImportError:
        pass
    mod = types.ModuleType("antenv.axon_hooks")
    mod._hook = None

    def set_axon_ntff_profile_hook(h):
        mod._hook = h

    def get_axon_ntff_profile_hook():
        if mod._hook is None:
            try:
                from trn_agent_boot.trn_boot import _ntff_profile_via_ctypes

                mod._hook = _ntff_profile_via_ctypes("/opt/axon/libaxon_pjrt.so")
            except Exception:
                mod._hook = None
        return mod._hook

    mod.set_axon_ntff_profile_hook = set_axon_ntff_profile_hook
    mod.get_axon_ntff_profile_hook = get_axon_ntff_profile_hook
    sys.modules["antenv.axon_hooks"] = mod
    antenv.axon_hooks = mod


_ensure_ntff_hook()

import concourse.bacc as bacc
import concourse.tile as tile
from concourse import mybir
from concourse.bass_utils import run_bass_kernel_spmd

T, D, I, E, TOPK = 2048, 1024, 4096, 8, 2
N_CORES = 8
CAP = 512            # tokens per expert per pass (multiple of 128, <=512)
DT = D // 128        # 8 contraction steps for layer 1
NI = I // 128        # 32 intermediate tiles
NQ = 8               # layer-2 w2 quads per d-half (4 i-tiles each)
F32 = mybir.dt.float32
BF16 = mybir.dt.bfloat16
BF16_NP = ml_dtypes.bfloat16
N_WARM = 68          # tiny matmuls to lift the HAM clock gate during the ramp

_NCS = {}            # compiled Bass modules, keyed by ncol
_WCACHE = {}         # packed per-expert weights, keyed on input identity
LAST_RESULTS = None  # BassKernelResults of the most recent SPMD run


def _build_nc(ncol, sim_act=False):
    # sim_act: CoreSim lacks Silu; emit sigmoid + extra multiply instead
    # (same math) so the program can be validated in simulation.
    nt = (ncol + 127) // 128          # token tiles
    last_rows = ncol - 128 * (nt - 1)  # rows in the final token tile
    nc = bacc.Bacc(
        "TRN2", target_bir_lowering=False, debug=False, num_devices=N_CORES
    )
    # Packed layouts (see _pack_weights): every DMA below reads HBM linearly.
    xt_d = nc.dram_tensor("xt", [DT, 128, CAP], BF16, kind="ExternalInput").ap()
    g_d = nc.dram_tensor("g", [CAP], F32, kind="ExternalInput").ap()
    w13p_d = nc.dram_tensor(
        "w13p", [NI, 2, 128, DT, 128], BF16, kind="ExternalInput"
    ).ap()
    w2p_d = nc.dram_tensor("w2p", [2, NI, 128, 512], BF16, kind="ExternalInput").ap()
    y_d = nc.dram_tensor("y", [CAP, D], F32, kind="ExternalOutput").ap()

    with tile.TileContext(nc) as tc:
        with (
            tc.tile_pool(name="consts", bufs=1) as const_pool,
            tc.tile_pool(name="w13", bufs=7) as w13_pool,
            tc.tile_pool(name="w2", bufs=10) as w2_pool,
            tc.tile_pool(name="h", bufs=1) as h_pool,
            tc.tile_pool(name="tmp", bufs=2) as tmp_pool,
            tc.tile_pool(name="yout", bufs=4) as out_pool,
        ):
            # PE warmup: the HAM clock gate needs ~3.4us of sustained PE
            # activity to move 1.2 -> 2.4 GHz. Run tiny matmuls on a memset
            # tile (no DMA dependency) while the first real tiles stream in.
            warm_sb = const_pool.tile([128, 64], BF16)
            nc.vector.memset(warm_sb[:], 0.0)
            warm_pool = tc.alloc_tile_pool(name="warm", bufs=1, space="PSUM")
            warm_ps = warm_pool.tile([64, 64], F32)
            for _ in range(N_WARM):
                nc.tensor.matmul(
                    warm_ps[:], warm_sb[:, :64], warm_sb[:, :64],
                    start=True, stop=True,
                )
            warm_pool.release()

            # Resident activations: x^T as 8 [128, CAP] d-tiles. The SDMA
            # engines round-robin all queued rings at packet granularity, so
            # aggregate HBM bandwidth (~300 GB/s) is the ramp constraint:
            # every ramp-critical transfer goes on the sync ring, in exact
            # consumption order ([xt lo, w13 it0, xt hi, w13 it1, it2, ...]),
            # so FIFO serialization gives each one full bandwidth in turn.
            xt_sb = const_pool.tile([128, DT, CAP], BF16)
            nc.sync.dma_start(xt_sb[:], xt_d.rearrange("a p c -> p a c"))
            g_sb = const_pool.tile([128, nt], F32)

            # hT[i, t] — layer-1 output (bf16), transposed so it is lhsT for
            # layer 2.
            hT = h_pool.tile([128, NI, ncol], BF16)

            # ps2 allocated first so ps1 can be released in LIFO order after
            # layer 1 while ps2's banks stay live for both layer-2 halves.
            ps2_pool = tc.alloc_tile_pool(name="ps2", bufs=1, space="PSUM")
            ps1_pool = tc.alloc_tile_pool(name="ps1", bufs=2, space="PSUM")
            w2_tiles = {}
            for it in range(NI):
                w13_t = w13_pool.tile([128, 2, DT, 128], BF16, tag="w13")
                # One linear 512 KB DMA per w13 tile. A ring's DMAs serialize
                # (transfer + ~1.4us completion receipt, ~2.8us per tile), so
                # w13 is split across gpsimd (it0, it1, odds — the ramp-
                # critical head) and sync (behind the 1 MB xt load). ACT/DVE
                # stay trigger-free so the silu/mul chain never blocks the PE.
                eng = nc.gpsimd if (it <= 1 or it % 2 == 1) else nc.sync
                eng.dma_start(
                    w13_t[:], w13p_d[it].rearrange("m p a c -> p m a c")
                )
                if it == NI - 1:
                    # Gates: tiny load, needed only at the layer-2 drain.
                    nc.gpsimd.dma_start(
                        g_sb[:], g_d[:nt * 128].rearrange("(a p) -> p a", p=128)
                    )
                if it >= 16 and it % 2 == 0:
                    # w2 dc0 quads prefetch on the idle gpsimd ring once the
                    # ramp is past; issuing them any earlier steals HBM
                    # bandwidth from the ramp-critical w13 stream.
                    iq = (it - 16) // 2
                    w2_t = w2_pool.tile([128, 4, 512], BF16, tag="w2")
                    nc.gpsimd.dma_start(
                        w2_t[:],
                        w2p_d[0, iq * 4:(iq + 1) * 4].rearrange("a p n -> p a n"),
                    )
                    w2_tiles[(0, iq)] = w2_t
                h1_ps = ps1_pool.tile([128, ncol], F32, tag="h1")
                h3_ps = ps1_pool.tile([128, ncol], F32, tag="h3")
                for dt_i in range(DT):
                    nc.tensor.matmul(
                        h1_ps[:],
                        w13_t[:, 0, dt_i, :],
                        xt_sb[:, dt_i, :ncol],
                        start=(dt_i == 0),
                        stop=(dt_i == DT - 1),
                    )
                for dt_i in range(DT):
                    nc.tensor.matmul(
                        h3_ps[:],
                        w13_t[:, 1, dt_i, :],
                        xt_sb[:, dt_i, :ncol],
                        start=(dt_i == 0),
                        stop=(dt_i == DT - 1),
                    )
                s_sb = tmp_pool.tile([128, ncol], F32)
                if sim_act:
                    nc.scalar.activation(
                        s_sb[:], h1_ps[:], mybir.ActivationFunctionType.Sigmoid
                    )
                    nc.vector.tensor_mul(s_sb[:], s_sb[:], h1_ps[:])
                else:
                    nc.scalar.activation(
                        s_sb[:], h1_ps[:], mybir.ActivationFunctionType.Silu
                    )
                nc.vector.tensor_mul(hT[:, it, :], s_sb[:], h3_ps[:])

            ps1_pool.release()

            # Layer 2, one d-half (512 cols) at a time: one PSUM bank per
            # token-tile accumulates y[t, dc] over the 32 i-tiles. The final
            # quad runs token-tile-major in 2+2 i-groups so each tile's drain
            # and output DMA overlap the remaining tiles' matmuls.
            def rows(tt):
                return last_rows if tt == nt - 1 else 128

            def l2mm(y_ps, tt, it, w2_t, j):
                nc.tensor.matmul(
                    y_ps[tt][:rows(tt), :],
                    hT[:, it, tt * 128:tt * 128 + rows(tt)],
                    w2_t[:, j, :],
                    start=(it == 0),
                    stop=(it == NI - 1),
                )

            def drain(dc, tt, y_ps):
                r = rows(tt)
                y_sb = out_pool.tile([128, 512], F32)
                src = y_ps[tt]
                gs = g_sb[:r, tt:tt + 1]
                lo, hi = (slice(0, 256), slice(256, 512))
                if tt % 2:
                    lo, hi = hi, lo
                nc.scalar.activation(
                    y_sb[:r, lo], src[:r, lo],
                    mybir.ActivationFunctionType.Copy, scale=gs,
                )
                nc.vector.tensor_scalar_mul(y_sb[:r, hi], src[:r, hi], gs)
                dst = y_d[tt * 128:tt * 128 + r, dc * 512:(dc + 1) * 512]
                if dc == 0:
                    eng = nc.gpsimd
                elif tt == nt - 1:
                    # Kernel-tail DMA rides the scalar ring, which has no
                    # other DMA and so no queued completion receipts.
                    eng = nc.scalar
                else:
                    eng = nc.gpsimd if tt % 2 == 0 else nc.sync
                eng.dma_start(dst, y_sb[:r, :])

            for dc in range(2):
                # One PSUM tile per token-tile: tile-granular dependency
                # tracking would otherwise serialize each tile's drain
                # against the next tile's matmuls.
                y_ps = [
                    ps2_pool.tile([128, 512], F32, tag=f"y{tt}", name=f"y_ps{tt}")
                    for tt in range(nt)
                ]
                for iq in range(NQ):
                    if dc == 0:
                        # Loaded during layer 1 on the gpsimd ring.
                        w2_t = w2_tiles[(0, iq)]
                    else:
                        # dc1 quads on sync: they queue behind the w13 loads
                        # and prefetch fully during dc0's matmuls.
                        w2_t = w2_pool.tile([128, 4, 512], BF16, tag="w2")
                        nc.sync.dma_start(
                            w2_t[:],
                            w2p_d[1, iq * 4:(iq + 1) * 4].rearrange("a p n -> p a n"),
                        )
                    if iq < NQ - 1:
                        for j in range(4):
                            for tt in range(nt):
                                l2mm(y_ps, tt, iq * 4 + j, w2_t, j)
                    else:
                        # Final quad: 2+2 i-groups, token-tile-major, with
                        # each tile's drain right after its last matmul.
                        for tt in range(nt):
                            for j in range(2):
                                l2mm(y_ps, tt, iq * 4 + j, w2_t, j)
                        for tt in range(nt):
                            for j in range(2, 4):
                                l2mm(y_ps, tt, iq * 4 + j, w2_t, j)
                            drain(dc, tt, y_ps)
            ps2_pool.release()

    nc.compile()
    return nc


def _pack_weights(w1, w2, w3):
    """Per-expert device layouts (bf16), all linear HBM reads:
    w13p[it, m, p, dt, c] = wm[it*128+c, dt*128+p]  (i.e. w.T tiled for lhsT)
    w2p[dc, it, p, n] = w2[dc*512+n, it*128+p]      (w2.T tiled by d-half)."""
    key = tuple((a.ctypes.data, a.shape) for a in (w1, w2, w3))
    if _WCACHE.get("key") == key:
        return _WCACHE["maps"]
    maps = []
    for e in range(E):
        w13p = np.empty((NI, 2, 128, DT, 128), dtype=BF16_NP)
        w13p[:, 0] = w1[e].reshape(NI, 128, DT, 128).transpose(0, 3, 2, 1)
        w13p[:, 1] = w3[e].reshape(NI, 128, DT, 128).transpose(0, 3, 2, 1)
        w2p = np.ascontiguousarray(
            w2[e].T.reshape(NI, 128, 2, 512).transpose(2, 0, 1, 3)
        ).astype(BF16_NP)
        maps.append({"w13p": w13p, "w2p": w2p})
    _WCACHE["key"] = key
    _WCACHE["maps"] = maps
    return maps


def kernel(x, expert_indices, expert_weights, w1, w2, w3):
    global LAST_RESULTS
    x = np.ascontiguousarray(np.asarray(x, dtype=np.float32))
    idx = np.asarray(expert_indices)
    ew = np.asarray(expert_weights, dtype=np.float32)
    w1 = np.ascontiguousarray(np.asarray(w1, dtype=np.float32))
    w2 = np.ascontiguousarray(np.asarray(w2, dtype=np.float32))
    w3 = np.ascontiguousarray(np.asarray(w3, dtype=np.float32))

    # Host routing: unique tokens per expert, with both top-k gate weights of a
    # token merged (a token picking the same expert twice gets the summed gate).
    tok_lists, gate_lists = [], []
    for e in range(E):
        m = idx == e
        sel = np.nonzero(m.any(axis=1))[0]
        tok_lists.append(sel)
        gate_lists.append((ew * m).sum(axis=1)[sel].astype(np.float32))

    weight_maps = _pack_weights(w1, w2, w3)

    n_max = max(len(s) for s in tok_lists)
    n_pass = max(1, math.ceil(n_max / CAP))
    # Matmul free dim: actual max tokens this pass (rounded to 8), so the PE
    # streams no padded columns. Compiled once per distinct ncol.
    ncol = min(CAP, -(-min(n_max, CAP) // 8) * 8)
    if ncol not in _NCS:
        _NCS[ncol] = _build_nc(ncol)
    nc_mod = _NCS[ncol]

    out = np.zeros((T, D), dtype=np.float32)
    trace = bool(os.environ.get("BASS_TRACE"))
    for p in range(n_pass):
        in_maps = []
        chunks = []
        for e in range(E):
            sel = tok_lists[e][p * CAP:(p + 1) * CAP]
            g = gate_lists[e][p * CAP:(p + 1) * CAP]
            chunks.append(sel)
            xt = np.zeros((D, CAP), dtype=np.float32)
            if len(sel):
                xt[:, :len(sel)] = x[sel].T
            xt = xt.astype(BF16_NP).reshape(DT, 128, CAP)
            g_pad = np.zeros((CAP,), dtype=np.float32)
            g_pad[:len(sel)] = g
            in_maps.append({"xt": xt, "g": g_pad, **weight_maps[e]})
        # Rare transient NRT_EXEC_UNIT_UNRECOVERABLE errors have been observed
        # on the first execution of a fresh NEFF; a straight retry recovers.
        last_exc = None
        for attempt in range(3):
            try:
                LAST_RESULTS = run_bass_kernel_spmd(
                    nc_mod, in_maps, core_ids=list(range(N_CORES)),
                    trace=trace and attempt == 0,
                )
                break
            except Exception as exc:  # noqa: BLE001
                last_exc = exc
                time.sleep(3)
        else:
            raise last_exc
        for e in range(E):
            sel = chunks[e]
            if len(sel):
                out[sel] += LAST_RESULTS.results[e]["y"][:len(sel)]
    return out
